# revision 14
# baseline (speedup 1.0000x reference)
"""Trainium2 Bass kernel for nn_BlurTensor: gaussian_filter(sigma=k_size) over
ALL axes of x (B=32, C=3, H=512, W=512) with 'symmetric' boundary.

Decomposition: the blur is the linear operator A0 (x) A1 (x) A2 (x) A3 applied
as mode products (one blur matrix per axis, built on host from k_size).
B and C fold into a single 96x96 Kronecker-product mixing matrix (96 <= 128
partitions), so the device does three matmul passes: H, W (banded), BC.

Sharding: H axis split into 8 x 64-row output slabs; each core receives a
104-row input slab (radius-20 halo), which makes all passes core-local.

v2 (all-fp16 device pipeline, PSUM accumulation stays fp32):
  - fp16 matmuls stream 1 cycle/row vs fp32's 4 (fp32 = 2 half-speed passes);
    fp16 input halves the dominant DMA traffic; device output is fp16 too
    (host upcasts) halving output DMA. End-to-end rel err ~6e-4 vs the 2e-2
    gate.
  - One merged consts tensor (fewer per-call PJRT args); x pre-transposed on
    host to [hin, B*C, W] so every input DMA line is contiguous.
  - PSUM evacuations paired (two banks per copy) to halve the per-op DVE/ACT
    fixed cost; copies alternate between Vector and Scalar engines.

Device pipeline per core (all intermediates SBUF-resident):
  pass H : out[w, (bc,h)] via lhsT = data tile [h'=104, w-chunk 128],
           rhs = A2_local^T [104, 64]  -> X1 [P:w(4x128), F:(wc,bc,h)]
  pass W : banded 512x512 matrix, 4 chunked matmuls accumulating into one
           PSUM bank per h (start=True on first clears has_written; the
           overlapping band writes then accumulate-or-overwrite per element)
  pass BC: Kronecker matrix (padded to 128 cols for fast-weight-load) as
           stationary weights, rhs = X2h [96, 512]
"""

import numpy as np

import concourse.bass as bass
import concourse.bacc as bacc
import concourse.mybir as mybir
from concourse.tile import TileContext
from concourse.bass_utils import run_bass_kernel_spmd

TRUNCATE = 4.0
N_CORES = 8
F32 = mybir.dt.float32
DEV_DT = mybir.dt.float16
DEV_NP = np.float16


def _gauss_kernel(sigma):
    # matches scipy/jax _gaussian_kernel1d in float32, like the reference
    radius = int(TRUNCATE * sigma + 0.5)
    x = np.arange(-radius, radius + 1, dtype=np.float32)
    w = np.exp(-0.5 * (x / sigma) ** 2).astype(np.float32)
    w = (w / w.sum(dtype=np.float32)).astype(np.float32)
    return w.astype(np.float64), radius


def _blur_matrix(L, w, radius):
    """(L, L) float64 operator: blur(v) = A @ v with symmetric padding."""
    I = np.eye(L, dtype=np.float64)
    Ipad = np.pad(I, ((radius, radius), (0, 0)), mode="symmetric")
    A = np.empty((L, L), dtype=np.float64)
    for i in range(L):
        A[i, :] = w @ Ipad[i : i + 2 * radius + 1, :]
    return A


def _build_program(B, C, H, W, hin, hs, radius, n_reps=1, unroll=1):
    """Build the SPMD Bass program (identical on all cores).

    n_reps > 1 wraps the whole pipeline in a device-side For_i that re-runs
    it n_reps times (same DRAM buffers). Used only for timing: the axon PJRT
    dispatch costs ~3.3 ms per call, so per-exec HW time is measured as the
    slope between two rep counts. unroll=2 emits two independent reps per
    loop body (double-buffered X1, repartitioned PSUM) so consecutive
    executions overlap — the steady-state back-to-back throughput."""
    from contextlib import ExitStack

    BC = B * C
    assert BC <= 128 and hin <= 128 and W % 128 == 0
    NJ = W // 128  # w' chunks
    BCP = 128  # mbct padded cols (bc_out) for fast-weight-load

    n_c = hin * hs + 128 * NJ * W + BC * BCP

    nc = bacc.Bacc("TRN2")
    xs = nc.dram_tensor("xs", [hin, BC, W], DEV_DT, kind="ExternalInput")
    consts = nc.dram_tensor("consts", [n_c], DEV_DT, kind="ExternalInput")
    out = nc.dram_tensor("out", [BC, hs, W], DEV_DT, kind="ExternalOutput")

    GRP = 8  # bc per psum/copy group
    LDG = 2  # load groups per DMA
    n_grp = BC // GRP
    HB = 8  # h rows per output stage/DMA

    def band(j):
        return max(0, 128 * j - radius), min(W, 128 * j + 128 + radius)

    def pass_h(cpool, x1pool, ldpool, ph, const_eng=None):
        """Stream the input + consts, contract h' (data-stationary matmuls).
        Returns (t_x1, t_a3t, t_mbct)."""
        const_eng = const_eng or nc.scalar
        o = 0
        t_a2lt = cpool.tile([hin, hs], DEV_DT, tag="a2")
        const_eng.dma_start(
            out=t_a2lt[:],
            in_=consts[o : o + hin * hs].rearrange("(p f) -> p f", p=hin),
        )
        o += hin * hs

        t_x1 = x1pool.tile([128, NJ, BC, hs], DEV_DT, tag="x1")

        # First load is a single 8-bc group so the PE starts early; the
        # rest stream LDG groups per DMA. a3t/mbct (only needed by pass
        # W/BC) load after the first x slab is in flight.
        load_plan = [1]
        while sum(load_plan) < n_grp:
            load_plan.append(min(LDG, n_grp - sum(load_plan)))
        g0 = 0
        t_a3t = t_mbct = None
        for li, ng in enumerate(load_plan):
            xt = ldpool.tile([hin, LDG, GRP, W], DEV_DT, tag="xt")
            nc.sync.dma_start(
                out=xt[:, :ng],
                in_=xs[:, g0 * GRP : (g0 + ng) * GRP, :].rearrange(
                    "h (l g) w -> h l g w", l=ng
                ),
            )
            if li == 0:
                t_a3t = cpool.tile([128, NJ, W], DEV_DT, tag="a3")
                const_eng.dma_start(
                    out=t_a3t[:],
                    in_=consts[o : o + 128 * NJ * W].rearrange(
                        "(p j n) -> p j n", p=128, j=NJ
                    ),
                )
                o += 128 * NJ * W
                t_mbct = cpool.tile([BC, BCP], DEV_DT, tag="mb")
                const_eng.dma_start(
                    out=t_mbct[:],
                    in_=consts[o : o + BC * BCP].rearrange("(p f) -> p f", p=BC),
                )
            for gg in range(ng):
                g = g0 + gg
                for jp in range(NJ // 2):
                    ps = ph.tile([128, 2, GRP * hs], F32, tag="ph")  # 2 banks
                    for jj in range(2):
                        j = 2 * jp + jj
                        for i in range(GRP):
                            nc.tensor.matmul(
                                ps[:, jj, i * hs : (i + 1) * hs],
                                lhsT=xt[:, gg, i, 128 * j : 128 * (j + 1)],
                                rhs=t_a2lt[:],
                                start=True,
                                stop=True,
                            )
                    dst = t_x1[:, 2 * jp : 2 * jp + 2, g * GRP : (g + 1) * GRP, :]
                    if (g * NJ // 2 + jp) % 2 == 0:
                        nc.vector.tensor_copy(dst, ps[:])
                    else:
                        nc.scalar.copy(dst, ps[:])
            g0 += ng
        return t_x1, t_a3t, t_mbct

    def pass_wbc(t_x1, t_a3t, t_mbct, x2pool, stpool, pw, pb, store_eng=None):
        """Banded W blur then BC mixing, h-paired and skewed."""
        store_eng = store_eng or nc.sync
        x2_tiles = {}
        stage = None

        def emit_w_pair(hp):
            ps = pw.tile([BC, 2, W], F32, tag="w")  # 2 banks
            for k in range(2):
                h = hp + k
                for j in range(NJ):
                    lo, hi = band(j)
                    nc.tensor.matmul(
                        ps[:, k, lo:hi],
                        lhsT=t_x1[:, j, :, h],
                        rhs=t_a3t[:, j, lo:hi],
                        start=(j == 0),
                        stop=(j == NJ - 1),
                    )
            x2 = x2pool.tile([BC, 2, W], DEV_DT, tag="x2")
            if (hp // 2) % 2 == 0:
                nc.vector.tensor_copy(x2[:], ps[:])
            else:
                nc.scalar.copy(x2[:], ps[:])
            x2_tiles[hp] = x2

        def emit_bc_pair(hp):
            nonlocal stage
            x2 = x2_tiles.pop(hp)
            ps2 = pb.tile([BCP, 2, W], F32, tag="b")  # 2 banks
            for k in range(2):
                nc.tensor.matmul(
                    ps2[:, k, :],
                    lhsT=t_mbct[:],
                    rhs=x2[:, k, :],
                    start=True,
                    stop=True,
                )
            hb, hr = divmod(hp, HB)
            if hr == 0:
                stage = stpool.tile([BC, HB, W], DEV_DT, tag="s")
            if (hp // 2) % 2 == 0:
                nc.scalar.copy(stage[:, hr : hr + 2, :], ps2[:BC])
            else:
                nc.vector.tensor_copy(stage[:, hr : hr + 2, :], ps2[:BC])
            if hr == HB - 2:
                store_eng.dma_start(
                    out=out[:, hb * HB : (hb + 1) * HB, :], in_=stage[:]
                )

        SKEW = 4  # h units (2 pairs)
        for hp in range(0, hs, 2):
            emit_w_pair(hp)
            if hp >= SKEW:
                emit_bc_pair(hp - SKEW)
        for hp in range(hs - SKEW, hs, 2):
            emit_bc_pair(hp)

    with TileContext(nc) as tc, ExitStack() as _st:
        if n_reps > unroll:
            assert n_reps % unroll == 0
            _st.enter_context(
                tc.For_i(
                    0, n_reps // unroll, 1, hint_engines=tuple(mybir.ALL_ENGINES)
                )
            )
        if unroll == 1:
            # Graded single-execution layout: pass-H PSUM pool (8 banks)
            # closes before the W/BC pools open.
            with (
                tc.tile_pool(name="const", bufs=1) as cpool,
                tc.tile_pool(name="x1p", bufs=1) as x1pool,
            ):
                with (
                    tc.tile_pool(name="ld", bufs=3) as ldpool,
                    tc.tile_pool(name="psh", bufs=4, space="PSUM") as ph,
                ):
                    t_x1, t_a3t, t_mbct = pass_h(cpool, x1pool, ldpool, ph)
                with (
                    tc.tile_pool(name="x2p", bufs=3) as x2pool,
                    tc.tile_pool(name="stg", bufs=2) as stpool,
                    tc.tile_pool(name="psw", bufs=2, space="PSUM") as pw,
                    tc.tile_pool(name="psb", bufs=2, space="PSUM") as pb,
                ):
                    pass_wbc(t_x1, t_a3t, t_mbct, x2pool, stpool, pw, pb)
        else:
            # Timing layout (steady-state throughput): SBUF pools (input
            # stream, X1, consts, staging) live across reps so rep u+1's
            # loads prefetch during rep u's W/BC phase; PSUM pools keep the
            # per-phase nested layout (full 8 banks each phase). Loads +
            # consts ride the sync HWDGE ring, stores the scalar ring, so
            # per-engine FIFO order never queues the next rep's input behind
            # this rep's stores.
            with (
                tc.tile_pool(name="const", bufs=2) as cpool,
                tc.tile_pool(name="x1p", bufs=2) as x1pool,
                tc.tile_pool(name="ld", bufs=3) as ldpool,
                tc.tile_pool(name="x2p", bufs=3) as x2pool,
                tc.tile_pool(name="stg", bufs=2) as stpool,
            ):
                for _u in range(unroll):
                    with tc.tile_pool(name="psh", bufs=4, space="PSUM") as ph:
                        t_x1, t_a3t, t_mbct = pass_h(
                            cpool, x1pool, ldpool, ph, const_eng=nc.sync
                        )
                    with (
                        tc.tile_pool(name="psw", bufs=2, space="PSUM") as pw,
                        tc.tile_pool(name="psb", bufs=2, space="PSUM") as pb,
                    ):
                        pass_wbc(
                            t_x1, t_a3t, t_mbct, x2pool, stpool, pw, pb,
                            store_eng=nc.scalar,
                        )
    nc.finalize()
    return nc


_CACHE = {}


def build_program_for(x_shape, k_size, n_reps=1, unroll=1):
    """Program with the same I/O contract as prepare()'s, optionally looped
    n_reps times on-device (timing use)."""
    B, C, H, W = x_shape
    w, radius = _gauss_kernel(float(k_size))
    hs = H // N_CORES
    hin = hs + 2 * radius
    return _build_program(
        B, C, H, W, hin, hs, radius, n_reps=n_reps, unroll=unroll
    )


def prepare(x, k_size):
    """Build (cached) program + per-core input maps for the given x."""
    x = np.ascontiguousarray(np.asarray(x, dtype=np.float32))
    B, C, H, W = x.shape
    BC = B * C
    sigma = float(k_size)
    w, radius = _gauss_kernel(sigma)

    hs = H // N_CORES
    hin = hs + 2 * radius
    assert hin <= min(H, 128), (hin, H)

    key = (B, C, H, W, sigma)
    if key not in _CACHE:
        A0 = _blur_matrix(B, w, radius)
        A1 = _blur_matrix(C, w, radius)
        A2 = _blur_matrix(H, w, radius)
        A3 = _blur_matrix(W, w, radius)

        # band-structure sanity: chunk j' of A3^T only reaches cols [lo, hi)
        A3T = A3.T
        for j in range(W // 128):
            lo, hi = max(0, 128 * j - radius), min(W, 128 * j + 128 + radius)
            assert np.abs(np.delete(A3T[128 * j : 128 * (j + 1)], np.s_[lo:hi], axis=1)).max() == 0.0

        # a3t in device layout [128, NJ, W] (partition-major, contiguous DMA)
        a3tp = np.ascontiguousarray(
            A3T.reshape(W // 128, 128, W).transpose(1, 0, 2).astype(DEV_NP)
        )
        mbct = np.kron(A0, A1).T.astype(DEV_NP)  # [bc_in 96, bc_out 96]
        mbct_pad = np.zeros((BC, 128), DEV_NP)
        mbct_pad[:, :BC] = mbct

        h0s, a2lts = [], []
        for m in range(N_CORES):
            h0 = min(max(hs * m - radius, 0), H - hin)
            rows = A2[hs * m : hs * (m + 1), :]
            mask = np.ones(H, bool)
            mask[h0 : h0 + hin] = False
            assert np.abs(rows[:, mask]).max() == 0.0, m
            h0s.append(h0)
            a2lts.append(
                np.ascontiguousarray(rows[:, h0 : h0 + hin].T.astype(DEV_NP))
            )

        consts = [
            np.concatenate(
                [a2lts[m].ravel(), a3tp.ravel(), mbct_pad.ravel()]
            ).astype(DEV_NP)
            for m in range(N_CORES)
        ]
        nc = _build_program(B, C, H, W, hin, hs, radius)
        _CACHE[key] = (nc, h0s, consts)

    nc, h0s, consts = _CACHE[key]

    x16 = x.astype(DEV_NP)
    in_maps = [
        {
            # [hin, BC, W]: h-slab, transposed so DMA lines are contiguous
            "xs": np.ascontiguousarray(
                x16[:, :, h0s[m] : h0s[m] + hin, :]
                .reshape(BC, hin, W)
                .transpose(1, 0, 2)
            ),
            "consts": consts[m],
        }
        for m in range(N_CORES)
    ]
    return nc, in_maps


def assemble(outs, B=32, C=3, H=512, W=512):
    """Per-core out tensors [BC, hs, W] (fp16) -> full [B, C, H, W] f32."""
    full = np.concatenate(outs, axis=1)
    return np.ascontiguousarray(full.astype(np.float32).reshape(B, C, H, W))


def kernel(x, k_size):
    x = np.ascontiguousarray(np.asarray(x, dtype=np.float32))
    B, C, H, W = x.shape
    nc, in_maps = prepare(x, k_size)
    res = run_bass_kernel_spmd(nc, in_maps, core_ids=list(range(N_CORES)))
    return assemble(
        [res.results[m]["out"] for m in range(N_CORES)], B, C, H, W
    )


# revision 24
# speedup vs baseline: 1.0206x; 1.0206x over previous
"""Trainium2 Bass kernel for nn_BlurTensor: gaussian_filter(sigma=k_size) over
ALL axes of x (B=32, C=3, H=512, W=512) with 'symmetric' boundary.

Decomposition: the blur is the linear operator A0 (x) A1 (x) A2 (x) A3 applied
as mode products (one blur matrix per axis, built on host from k_size).
B and C fold into a single 96x96 Kronecker-product mixing matrix (96 <= 128
partitions), so the device does three matmul passes: H, W (banded), BC.

Sharding: H axis split into 8 x 64-row output slabs; each core receives a
104-row input slab (radius-20 halo), which makes all passes core-local.

v2 (all-fp16 device pipeline, PSUM accumulation stays fp32):
  - fp16 matmuls stream 1 cycle/row vs fp32's 4 (fp32 = 2 half-speed passes);
    fp16 input halves the dominant DMA traffic; device output is fp16 too
    (host upcasts) halving output DMA. End-to-end rel err ~6e-4 vs the 2e-2
    gate.
  - One merged consts tensor (fewer per-call PJRT args); x pre-transposed on
    host to [hin, B*C, W] so every input DMA line is contiguous.
  - PSUM evacuations paired (two banks per copy) to halve the per-op DVE/ACT
    fixed cost; copies alternate between Vector and Scalar engines.

Device pipeline per core (all intermediates SBUF-resident):
  pass H : out[w, (bc,h)] via lhsT = data tile [h'=104, w-chunk 128],
           rhs = A2_local^T [104, 64]  -> X1 [P:w(4x128), F:(wc,bc,h)]
  pass W : banded 512x512 matrix, 4 chunked matmuls accumulating into one
           PSUM bank per h (start=True on first clears has_written; the
           overlapping band writes then accumulate-or-overwrite per element)
  pass BC: Kronecker matrix (padded to 128 cols for fast-weight-load) as
           stationary weights, rhs = X2h [96, 512]
"""

import numpy as np

import concourse.bass as bass
import concourse.bacc as bacc
import concourse.mybir as mybir
from concourse.tile import TileContext
from concourse.bass_utils import run_bass_kernel_spmd

TRUNCATE = 4.0
N_CORES = 8
F32 = mybir.dt.float32
DEV_DT = mybir.dt.float16
DEV_NP = np.float16


def _gauss_kernel(sigma):
    # matches scipy/jax _gaussian_kernel1d in float32, like the reference
    radius = int(TRUNCATE * sigma + 0.5)
    x = np.arange(-radius, radius + 1, dtype=np.float32)
    w = np.exp(-0.5 * (x / sigma) ** 2).astype(np.float32)
    w = (w / w.sum(dtype=np.float32)).astype(np.float32)
    return w.astype(np.float64), radius


def _blur_matrix(L, w, radius):
    """(L, L) float64 operator: blur(v) = A @ v with symmetric padding."""
    I = np.eye(L, dtype=np.float64)
    Ipad = np.pad(I, ((radius, radius), (0, 0)), mode="symmetric")
    A = np.empty((L, L), dtype=np.float64)
    for i in range(L):
        A[i, :] = w @ Ipad[i : i + 2 * radius + 1, :]
    return A


def _build_program(B, C, H, W, hin, hs, radius, n_reps=1, unroll=1, staggered=False):
    """Build the SPMD Bass program (identical on all cores).

    n_reps > 1 wraps the whole pipeline in a device-side For_i that re-runs
    it n_reps times (same DRAM buffers). Used only for timing: the axon PJRT
    dispatch costs ~3.3 ms per call, so per-exec HW time is measured as the
    slope between two rep counts. unroll=2 emits two independent reps per
    loop body (double-buffered X1, repartitioned PSUM) so consecutive
    executions overlap — the steady-state back-to-back throughput."""
    from contextlib import ExitStack

    BC = B * C
    assert BC <= 128 and hin <= 128 and W % 128 == 0
    NJ = W // 128  # w' chunks
    BCP = 128  # mbct padded cols (bc_out) for fast-weight-load

    n_c = hin * hs + 128 * NJ * W + BC * BCP

    nc = bacc.Bacc("TRN2")
    xs = nc.dram_tensor("xs", [hin, BC, W], DEV_DT, kind="ExternalInput")
    consts = nc.dram_tensor("consts", [n_c], DEV_DT, kind="ExternalInput")
    out = nc.dram_tensor("out", [BC, hs, W], DEV_DT, kind="ExternalOutput")

    GRP = 8  # bc per psum/copy group
    LDG = 2  # load groups per DMA
    n_grp = BC // GRP
    HB = 8  # h rows per output stage/DMA

    def band(j):
        return max(0, 128 * j - radius), min(W, 128 * j + 128 + radius)

    def load_consts(cpool, const_eng):
        o = 0
        t_a2lt = cpool.tile([hin, hs], DEV_DT, tag="a2")
        const_eng.dma_start(
            out=t_a2lt[:],
            in_=consts[o : o + hin * hs].rearrange("(p f) -> p f", p=hin),
        )
        o += hin * hs
        t_a3t = cpool.tile([128, NJ, W], DEV_DT, tag="a3")
        const_eng.dma_start(
            out=t_a3t[:],
            in_=consts[o : o + 128 * NJ * W].rearrange(
                "(p j n) -> p j n", p=128, j=NJ
            ),
        )
        o += 128 * NJ * W
        t_mbct = cpool.tile([BC, BCP], DEV_DT, tag="mb")
        const_eng.dma_start(
            out=t_mbct[:],
            in_=consts[o : o + BC * BCP].rearrange("(p f) -> p f", p=BC),
        )
        return t_a2lt, t_a3t, t_mbct

    def load_plan_fn():
        load_plan = [1]
        while sum(load_plan) < n_grp:
            load_plan.append(min(LDG, n_grp - sum(load_plan)))
        return load_plan

    def issue_load(ldpool, li_g0_ng):
        li, g0, ng = li_g0_ng
        xt = ldpool.tile([hin, LDG, GRP, W], DEV_DT, tag=f"xt{li}")
        nc.sync.dma_start(
            out=xt[:, :ng],
            in_=xs[:, g0 * GRP : (g0 + ng) * GRP, :].rearrange(
                "h (l g) w -> h l g w", l=ng
            ),
        )
        return xt

    def pass_h_compute(t_a2lt, t_x1, ph, xt, g0, ng, ps_tag="ph"):
        for gg in range(ng):
            g = g0 + gg
            for jp in range(NJ // 2):
                ps = ph.tile([128, 2, GRP * hs], F32, tag=ps_tag)  # 2 banks
                for jj in range(2):
                    j = 2 * jp + jj
                    for i in range(GRP):
                        nc.tensor.matmul(
                            ps[:, jj, i * hs : (i + 1) * hs],
                            lhsT=xt[:, gg, i, 128 * j : 128 * (j + 1)],
                            rhs=t_a2lt[:],
                            start=True,
                            stop=True,
                        )
                dst = t_x1[:, 2 * jp : 2 * jp + 2, g * GRP : (g + 1) * GRP, :]
                if (g * NJ // 2 + jp) % 2 == 0:
                    nc.vector.tensor_copy(dst, ps[:])
                else:
                    nc.scalar.copy(dst, ps[:])

    def pass_h(cpool, x1pool, ldpool, ph, const_eng=None):
        """Stream the input + consts, contract h' (data-stationary matmuls).
        Loads interleave with compute (graded layout: a2lt first, the rest
        of the consts after the first x slab is in flight).
        Returns (t_x1, t_a3t, t_mbct)."""
        const_eng = const_eng or nc.scalar
        o = 0
        t_a2lt = cpool.tile([hin, hs], DEV_DT, tag="a2")
        const_eng.dma_start(
            out=t_a2lt[:],
            in_=consts[o : o + hin * hs].rearrange("(p f) -> p f", p=hin),
        )
        o += hin * hs

        t_x1 = x1pool.tile([128, NJ, BC, hs], DEV_DT, tag="x1")

        g0 = 0
        t_a3t = t_mbct = None
        for li, ng in enumerate(load_plan_fn()):
            xt = ldpool.tile([hin, LDG, GRP, W], DEV_DT, tag="xt")
            nc.sync.dma_start(
                out=xt[:, :ng],
                in_=xs[:, g0 * GRP : (g0 + ng) * GRP, :].rearrange(
                    "h (l g) w -> h l g w", l=ng
                ),
            )
            if li == 0:
                t_a3t = cpool.tile([128, NJ, W], DEV_DT, tag="a3")
                const_eng.dma_start(
                    out=t_a3t[:],
                    in_=consts[o : o + 128 * NJ * W].rearrange(
                        "(p j n) -> p j n", p=128, j=NJ
                    ),
                )
                o += 128 * NJ * W
                t_mbct = cpool.tile([BC, BCP], DEV_DT, tag="mb")
                const_eng.dma_start(
                    out=t_mbct[:],
                    in_=consts[o : o + BC * BCP].rearrange("(p f) -> p f", p=BC),
                )
            pass_h_compute(t_a2lt, t_x1, ph, xt, g0, ng)
            g0 += ng
        return t_x1, t_a3t, t_mbct

    def pass_wbc(
        t_x1, t_a3t, t_mbct, x2pool, stpool, pw, pb, store_eng=None,
        mid_cb=None, mid_hp=None, w_tag="w", b_tag="b",
    ):
        """Banded W blur then BC mixing, h-paired and skewed."""
        store_eng = store_eng or nc.sync
        x2_tiles = {}
        stage = None

        def emit_w_pair(hp):
            ps = pw.tile([BC, 2, W], F32, tag=w_tag)  # 2 banks
            for k in range(2):
                h = hp + k
                for j in range(NJ):
                    lo, hi = band(j)
                    nc.tensor.matmul(
                        ps[:, k, lo:hi],
                        lhsT=t_x1[:, j, :, h],
                        rhs=t_a3t[:, j, lo:hi],
                        start=(j == 0),
                        stop=(j == NJ - 1),
                    )
            x2 = x2pool.tile([BC, 2, W], DEV_DT, tag="x2")
            if (hp // 2) % 2 == 0:
                nc.vector.tensor_copy(x2[:], ps[:])
            else:
                nc.scalar.copy(x2[:], ps[:])
            x2_tiles[hp] = x2

        def emit_bc_pair(hp):
            nonlocal stage
            x2 = x2_tiles.pop(hp)
            ps2 = pb.tile([BCP, 2, W], F32, tag=b_tag)  # 2 banks
            for k in range(2):
                nc.tensor.matmul(
                    ps2[:, k, :],
                    lhsT=t_mbct[:],
                    rhs=x2[:, k, :],
                    start=True,
                    stop=True,
                )
            hb, hr = divmod(hp, HB)
            if hr == 0:
                stage = stpool.tile([BC, HB, W], DEV_DT, tag="s")
            if (hp // 2) % 2 == 0:
                nc.scalar.copy(stage[:, hr : hr + 2, :], ps2[:BC])
            else:
                nc.vector.tensor_copy(stage[:, hr : hr + 2, :], ps2[:BC])
            if hr == HB - 2:
                store_eng.dma_start(
                    out=out[:, hb * HB : (hb + 1) * HB, :], in_=stage[:]
                )

        SKEW = 4  # h units (2 pairs)
        for hp in range(0, hs, 2):
            if mid_cb is not None and hp == mid_hp:
                mid_cb()
            emit_w_pair(hp)
            if hp >= SKEW:
                emit_bc_pair(hp - SKEW)
        for hp in range(hs - SKEW, hs, 2):
            emit_bc_pair(hp)

    with TileContext(nc) as tc, ExitStack() as _st:
        if staggered:
            # Timing-only: staggered-reset loop — no all-engine back-edge
            # barrier; the 4-stage rotation lets iteration i+1's S0 (all
            # input loads, on the sync ring whose later stages are empty)
            # issue while iteration i finishes its last W/BC stage. All
            # pools (incl. one shared 4-slot x 2-bank PSUM pool = 8 banks)
            # open BEFORE the loop: pool-boundary instructions cannot live
            # inside a staggered body.
            assert n_reps > 1 and unroll == 1
            cpool = _st.enter_context(tc.tile_pool(name="const", bufs=1))
            x1pool = _st.enter_context(tc.tile_pool(name="x1p", bufs=1))
            ldpool = _st.enter_context(tc.tile_pool(name="ld", bufs=1))
            x2pool = _st.enter_context(tc.tile_pool(name="x2p", bufs=3))
            stpool = _st.enter_context(tc.tile_pool(name="stg", bufs=2))
            psa = _st.enter_context(tc.tile_pool(name="psa", bufs=4, space="PSUM"))
            _st.enter_context(
                tc.For_i(
                    0, n_reps, 1,
                    staggered_reset=True,
                    hint_engines=tuple(mybir.ALL_ENGINES),
                )
            )
            if True:
                # S0: consts + every input load issued up front
                t_a2lt, t_a3t, t_mbct = load_consts(cpool, nc.sync)
                xts = []
                g0 = 0
                for li, ng in enumerate(load_plan_fn()):
                    xts.append((issue_load(ldpool, (li, g0, ng)), g0, ng))
                    g0 += ng
                tc.stage_boundary()
                # S1: pass-H compute
                t_x1 = x1pool.tile([128, NJ, BC, hs], DEV_DT, tag="x1")
                for xt, gg0, ng in xts:
                    pass_h_compute(t_a2lt, t_x1, psa, xt, gg0, ng, ps_tag="ps")
                tc.stage_boundary()
                # S2 / S3: W/BC halves (third boundary mid-loop); stores on
                # the scalar ring so SP's S1-S3 stay empty
                pass_wbc(
                    t_x1, t_a3t, t_mbct, x2pool, stpool, psa, psa,
                    store_eng=nc.scalar,
                    mid_cb=tc.stage_boundary, mid_hp=16,
                    w_tag="ps", b_tag="ps",
                )
        elif n_reps > unroll:
            assert n_reps % unroll == 0
            _st.enter_context(
                tc.For_i(
                    0, n_reps // unroll, 1, hint_engines=tuple(mybir.ALL_ENGINES)
                )
            )
        if staggered:
            pass  # body already emitted above
        elif unroll == 1:
            # Graded single-execution layout: pass-H PSUM pool (8 banks)
            # closes before the W/BC pools open.
            with (
                tc.tile_pool(name="const", bufs=1) as cpool,
                tc.tile_pool(name="x1p", bufs=1) as x1pool,
            ):
                with (
                    tc.tile_pool(name="ld", bufs=3) as ldpool,
                    tc.tile_pool(name="psh", bufs=4, space="PSUM") as ph,
                ):
                    t_x1, t_a3t, t_mbct = pass_h(cpool, x1pool, ldpool, ph)
                with (
                    tc.tile_pool(name="x2p", bufs=3) as x2pool,
                    tc.tile_pool(name="stg", bufs=2) as stpool,
                    tc.tile_pool(name="psw", bufs=2, space="PSUM") as pw,
                    tc.tile_pool(name="psb", bufs=2, space="PSUM") as pb,
                ):
                    pass_wbc(t_x1, t_a3t, t_mbct, x2pool, stpool, pw, pb)
        else:
            # Timing layout (steady-state throughput): SBUF pools (input
            # stream, X1, consts, staging) live across reps so rep u+1's
            # loads prefetch during rep u's W/BC phase; PSUM pools keep the
            # per-phase nested layout (full 8 banks each phase). Loads +
            # consts ride the sync HWDGE ring, stores the scalar ring, so
            # per-engine FIFO order never queues the next rep's input behind
            # this rep's stores.
            with (
                tc.tile_pool(name="const", bufs=2) as cpool,
                tc.tile_pool(name="x1p", bufs=2) as x1pool,
                tc.tile_pool(name="ld", bufs=3) as ldpool,
                tc.tile_pool(name="x2p", bufs=3) as x2pool,
                tc.tile_pool(name="stg", bufs=2) as stpool,
            ):
                for _u in range(unroll):
                    with tc.tile_pool(name="psh", bufs=4, space="PSUM") as ph:
                        t_x1, t_a3t, t_mbct = pass_h(
                            cpool, x1pool, ldpool, ph, const_eng=nc.sync
                        )
                    with (
                        tc.tile_pool(name="psw", bufs=2, space="PSUM") as pw,
                        tc.tile_pool(name="psb", bufs=2, space="PSUM") as pb,
                    ):
                        pass_wbc(
                            t_x1, t_a3t, t_mbct, x2pool, stpool, pw, pb,
                            store_eng=nc.scalar,
                        )
    nc.finalize()
    return nc


_CACHE = {}


def build_program_for(x_shape, k_size, n_reps=1, unroll=1, staggered=False):
    """Program with the same I/O contract as prepare()'s, optionally looped
    n_reps times on-device (timing use)."""
    B, C, H, W = x_shape
    w, radius = _gauss_kernel(float(k_size))
    hs = H // N_CORES
    hin = hs + 2 * radius
    return _build_program(
        B, C, H, W, hin, hs, radius, n_reps=n_reps, unroll=unroll,
        staggered=staggered,
    )


def prepare(x, k_size):
    """Build (cached) program + per-core input maps for the given x."""
    x = np.ascontiguousarray(np.asarray(x, dtype=np.float32))
    B, C, H, W = x.shape
    BC = B * C
    sigma = float(k_size)
    w, radius = _gauss_kernel(sigma)

    hs = H // N_CORES
    hin = hs + 2 * radius
    assert hin <= min(H, 128), (hin, H)

    key = (B, C, H, W, sigma)
    if key not in _CACHE:
        A0 = _blur_matrix(B, w, radius)
        A1 = _blur_matrix(C, w, radius)
        A2 = _blur_matrix(H, w, radius)
        A3 = _blur_matrix(W, w, radius)

        # band-structure sanity: chunk j' of A3^T only reaches cols [lo, hi)
        A3T = A3.T
        for j in range(W // 128):
            lo, hi = max(0, 128 * j - radius), min(W, 128 * j + 128 + radius)
            assert np.abs(np.delete(A3T[128 * j : 128 * (j + 1)], np.s_[lo:hi], axis=1)).max() == 0.0

        # a3t in device layout [128, NJ, W] (partition-major, contiguous DMA)
        a3tp = np.ascontiguousarray(
            A3T.reshape(W // 128, 128, W).transpose(1, 0, 2).astype(DEV_NP)
        )
        mbct = np.kron(A0, A1).T.astype(DEV_NP)  # [bc_in 96, bc_out 96]
        mbct_pad = np.zeros((BC, 128), DEV_NP)
        mbct_pad[:, :BC] = mbct

        h0s, a2lts = [], []
        for m in range(N_CORES):
            h0 = min(max(hs * m - radius, 0), H - hin)
            rows = A2[hs * m : hs * (m + 1), :]
            mask = np.ones(H, bool)
            mask[h0 : h0 + hin] = False
            assert np.abs(rows[:, mask]).max() == 0.0, m
            h0s.append(h0)
            a2lts.append(
                np.ascontiguousarray(rows[:, h0 : h0 + hin].T.astype(DEV_NP))
            )

        consts = [
            np.concatenate(
                [a2lts[m].ravel(), a3tp.ravel(), mbct_pad.ravel()]
            ).astype(DEV_NP)
            for m in range(N_CORES)
        ]
        nc = _build_program(B, C, H, W, hin, hs, radius)
        _CACHE[key] = (nc, h0s, consts)

    nc, h0s, consts = _CACHE[key]

    x16 = x.astype(DEV_NP)
    in_maps = [
        {
            # [hin, BC, W]: h-slab, transposed so DMA lines are contiguous
            "xs": np.ascontiguousarray(
                x16[:, :, h0s[m] : h0s[m] + hin, :]
                .reshape(BC, hin, W)
                .transpose(1, 0, 2)
            ),
            "consts": consts[m],
        }
        for m in range(N_CORES)
    ]
    return nc, in_maps


def assemble(outs, B=32, C=3, H=512, W=512):
    """Per-core out tensors [BC, hs, W] (fp16) -> full [B, C, H, W] f32."""
    full = np.concatenate(outs, axis=1)
    return np.ascontiguousarray(full.astype(np.float32).reshape(B, C, H, W))


def kernel(x, k_size):
    x = np.ascontiguousarray(np.asarray(x, dtype=np.float32))
    B, C, H, W = x.shape
    nc, in_maps = prepare(x, k_size)
    res = run_bass_kernel_spmd(nc, in_maps, core_ids=list(range(N_CORES)))
    return assemble(
        [res.results[m]["out"] for m in range(N_CORES)], B, C, H, W
    )
